# revision 11
# baseline (speedup 1.0000x reference)
"""Nearest-E8-lattice quantizer (CachedE8Quantizer) as a Bass/Tile kernel on 8 trn2 cores.

Input x: [8388608, 8] fp32. Output: nearest point of E8 = D8 u (D8 + 1/2).

Sharding: data-parallel over the points dim, 1/8 per core (no comms).

Math (per 8-vector), all derived from ONE rounding r0 = RNE(x), d0 = x - r0:
  branch0 (D8):    y0 = r0 (+ parity flip at argmax |d0| toward x)
  branch1 (D8+.5): r1h = r0 + 0.5*sgn(d0); |d1| = 0.5 - |d0|
  With a = |d0|, ma = max a, na = min a, sa = sum a, p0 = parity(sum r0),
  p1 = parity(sum r0 + #(d0>=0)):
    D0 - D1 = p0*(1 - 2*ma) + sa - 2 - 2*p1*na   (squared-dist sums cancel!)
    c = (D0 <= D1)  ->  branch 0
  Flip coordinate: branch0 -> a == ma, branch1 -> a == na; direction
  sgn(d0) for branch0, -sgn(d0) for branch1.
  y = r0 + sigma*(oh*flip_b + hb_b), sigma = 2*(d0>=0)-1,
    flip_b = c ? +1 : -1,  hb_b = c ? 0 : 0.5
  All of r0, oh, flips, halves are bf16-exact; y (half-integers < 16) is
  written as bf16 and converted to fp32 on the host (exact).

Layout per tile: [128 partitions, tf points, 8 coords]; segmented reduces
along the free axis; Pool pre-pairs coords 0:4 with 4:8 to halve DVE
reduce input.
"""

import numpy as np

from concourse import bacc
import concourse.mybir as mybir
from concourse.alu_op_type import AluOpType as op
from concourse.tile import TileContext

N_POINTS = 8388608
N_CORES = 8
SHARD = N_POINTS // N_CORES  # 1048576 points per core

MAGIC = 12582912.0  # 1.5 * 2**23: (x + MAGIC) - MAGIC == round-half-even(x)
F32 = mybir.dt.float32
BF16 = mybir.dt.bfloat16
F16 = mybir.dt.float16
U32 = mybir.dt.uint32
X = mybir.AxisListType.X

TF = 256

# engine routing (ablation knobs): "vector" = DVE, "gpsimd" = Pool, "scalar" = ACT
# NOTE Pool (gpsimd) only supports tt add/sub/mult and ts ops on TRN2 —
# no stt, no max/min/compare tt ops, no copy_predicated.
ENG = {
    "a1": "vector",    # r0b = round(x) -> bf16            (ts)
    "a2": "gpsimd",    # d0 = x - r0                       (tt sub)
    "a3": "gpsimd",    # npos = (d0 >= 0) -> bf16          (ts is_ge)
    "pp_rr": "gpsimd", # pre-pair sums of [r0|npos]        (tt add)
    "pp_sum": "gpsimd",# pre-pair sum of |d|               (tt add)
    "d1": "vector",    # oh = (a == mgq_b)                 (tt is_equal)
    "z1": "gpsimd",    # oh *= flip_b                      (tt mult)
    "z2": "gpsimd",    # z2 = z1 + hb_b                    (tt add)
    "z3": "vector",    # z3 = z2 * npos                    (tt mult)
    "w": "vector",     # w = 2*z3 - z2                     (stt)
    "y": "vector",     # y = r0 + w                        (tt add)
    "sm_tt": "gpsimd", # group-stage add/sub/mult tt ops
    "sm_ts": "vector", # group-stage ts ops
    "sm_stt": "vector", # group-stage stt ops (DVE only)
    "sm_cmp": "vector", # group-stage compare ops (DVE only)
}
PP_RR = True   # pre-pair [r0|npos] on Pool before DVE reduce
PP_SUM = True  # pre-pair |d| sum on Pool before DVE reduce


def _emit_tile(nc, pools, xd, yd, t, tf):
    E = lambda k: getattr(nc, ENG[k])
    P = 128
    pts = P * tf
    FE = tf * 8
    T = tf
    stream, work, small = pools

    s = t * pts
    x_rows = xd[s : s + pts, :].rearrange("(p f) c -> p (f c)", p=P)
    y_rows = yd[s : s + pts, :].rearrange("(p f) c -> p (f c)", p=P)

    xt = stream.tile([P, FE], F32, tag="xt")
    nc.sync.dma_start(out=xt[:], in_=x_rows)

    # rr = [r0 | npos] bf16
    rr = work.tile([P, 2 * FE], BF16, tag="rr")
    r0b = rr[:, :FE]
    nposb = rr[:, FE:]
    E("a1").tensor_scalar(r0b, xt[:], MAGIC, MAGIC, op0=op.add, op1=op.subtract)

    dd = work.tile([P, FE], F32, tag="dd")
    E("a2").tensor_tensor(dd[:], xt[:], r0b, op.subtract)
    E("a3").tensor_scalar(nposb, dd[:], 0.0, None, op0=op.is_ge)
    aa = work.tile([P, FE], F32, tag="aa")
    nc.scalar.activation(aa[:], dd[:], mybir.ActivationFunctionType.Abs)

    rr3 = rr[:].rearrange("p (t c) -> p t c", c=8)  # [P, 2T, 8]
    aa3 = aa[:].rearrange("p (t c) -> p t c", c=8)  # [P, T, 8]

    # group-stage arena
    ar = small.tile([P, 10 * T], F32, tag="ar")
    sboth = ar[:, 0 * T : 2 * T]   # [s0 | s1]
    uu = ar[:, 2 * T : 4 * T]      # parity scratch
    pp = ar[:, 4 * T : 6 * T]      # [p0 | p1]
    ma = ar[:, 6 * T : 7 * T]
    na = ar[:, 7 * T : 8 * T]
    sa = ar[:, 8 * T : 9 * T]
    e1 = ar[:, 9 * T : 10 * T]
    arb = small.tile([P, 2 * T], BF16, tag="arb")
    flipb = arb[:, 0 * T : 1 * T]
    hbb = arb[:, 1 * T : 2 * T]
    sm2 = small.tile([P, 2 * T], F32, tag="sm2")
    msel = sm2[:, :T]
    psel = sm2[:, T:]
    ssh = small.tile([P, 2 * T], F16, tag="ssh")

    if PP_RR:
        prs = work.tile([P, 8 * T], BF16, tag="prs")
        prs3 = prs[:].rearrange("p (t c) -> p t c", c=4)  # [P, 2T, 4]
        E("pp_rr").tensor_tensor(prs3, rr3[:, :, 0:4], rr3[:, :, 4:8], op.add)
        with nc.allow_low_precision(reason="sums of small ints, exact in f16"):
            nc.vector.tensor_reduce(ssh[:], prs3, axis=X, op=op.add)
    else:
        with nc.allow_low_precision(reason="sums of small ints, exact in f16"):
            nc.vector.tensor_reduce(ssh[:], rr3, axis=X, op=op.add)
    nc.vector.tensor_reduce(ma, aa3, axis=X, op=op.max)
    nc.vector.tensor_reduce(na, aa3, axis=X, op=op.min)
    if PP_SUM:
        prm = work.tile([P, 4 * T], F32, tag="prm")
        psm = prm[:].rearrange("p (t c) -> p t c", c=4)
        E("pp_sum").tensor_tensor(psm, aa3[:, :, 0:4], aa3[:, :, 4:8], op.add)
        nc.vector.tensor_reduce(sa, psm, axis=X, op=op.add)
    else:
        nc.vector.tensor_reduce(sa, aa3, axis=X, op=op.add)

    # ---- group stage ----
    # s0 (f32) and s1 = s0 + npos_sum
    E("sm_ts").tensor_copy(sboth[:, :T], ssh[:, :T])
    E("sm_tt").tensor_tensor(sboth[:, T:], ssh[:, :T], ssh[:, T:], op.add)
    # parity of [s0|s1]: p = (2*round(s/2) - s)^2
    E("sm_ts").tensor_scalar(uu, sboth, 0.5, MAGIC, op0=op.mult, op1=op.add)
    E("sm_ts").tensor_scalar(uu, uu, MAGIC, 2.0, op0=op.subtract, op1=op.mult)
    E("sm_tt").tensor_tensor(uu, uu, sboth, op.subtract)
    E("sm_tt").tensor_tensor(pp, uu, uu, op.mult)
    p0 = pp[:, :T]
    p1 = pp[:, T:]
    # e1 = p0*(1-2ma) + (sa-2) ; e2 = 2*na*p1 ; c = e1 <= e2
    E("sm_ts").tensor_scalar(e1, ma, -2.0, 1.0, op0=op.mult, op1=op.add)
    E("sm_tt").tensor_tensor(e1, e1, p0, op.mult)
    E("sm_stt").scalar_tensor_tensor(e1, sa, 2.0, e1, op0=op.subtract, op1=op.add)
    e2 = uu[:, :T]  # reuse
    E("sm_stt").scalar_tensor_tensor(e2, na, 2.0, p1, op0=op.mult, op1=op.mult)
    cf = uu[:, T:]  # reuse: c as f32
    E("sm_cmp").tensor_tensor(cf, e1, e2, op.is_le)
    # msel = c ? ma : na ; psel = c ? p0 : p1
    cfu = cf.bitcast(U32)
    nc.vector.tensor_copy(msel, na)
    nc.vector.copy_predicated(msel, cfu, ma)
    nc.vector.tensor_copy(psel, p1)
    nc.vector.copy_predicated(psel, cfu, p0)
    # mgq = psel*msel + (psel-1)   (== msel when psel=1, -1 when psel=0)
    mgq = e1  # reuse
    E("sm_tt").tensor_tensor(mgq, psel, msel, op.mult)
    E("sm_stt").scalar_tensor_tensor(mgq, psel, 1.0, mgq, op0=op.subtract, op1=op.add)
    # flip_b = 2c-1 in {+1,-1}; hb_b = c ? 0 : 0.5   (bf16)
    E("sm_ts").tensor_scalar(flipb, cf, 2.0, 1.0, op0=op.mult, op1=op.subtract)
    E("sm_ts").tensor_scalar(hbb, cf, -0.5, 0.5, op0=op.mult, op1=op.add)

    # ---- elementwise finish ----
    ohb = work.tile([P, FE], BF16, tag="ohb")
    ohb3 = ohb[:].rearrange("p (t c) -> p t c", c=8)
    mgq_b = mgq.unsqueeze(2).broadcast_to([P, T, 8])
    E("d1").tensor_tensor(ohb3, aa3, mgq_b, op.is_equal)
    flip_bb = flipb.unsqueeze(2).broadcast_to([P, T, 8])
    hb_bb = hbb.unsqueeze(2).broadcast_to([P, T, 8])
    E("z1").tensor_tensor(ohb3, ohb3, flip_bb, op.mult)
    E("z2").tensor_tensor(ohb3, ohb3, hb_bb, op.add)
    # z3 = z2 * npos (into npos slot); w = 2*z3 - z2 (into ohb); y = r0 + w
    E("z3").tensor_tensor(nposb, ohb[:], nposb, op.mult)
    E("w").scalar_tensor_tensor(ohb[:], nposb, 2.0, ohb[:], op0=op.mult, op1=op.subtract)
    ybb = stream.tile([P, FE], BF16, tag="ybb")
    E("y").tensor_tensor(ybb[:], r0b, ohb[:], op.add)
    nc.sync.dma_start(out=y_rows, in_=ybb[:])


def build_nc(shard=SHARD, tf=None, reps=1):
    if tf is None:
        tf = TF
    P = 128
    pts = P * tf
    assert shard % pts == 0
    ntiles = shard // pts

    nc = bacc.Bacc("TRN2", target_bir_lowering=False, debug=False, num_devices=N_CORES)
    xd = nc.declare_dram_parameter("x", [shard, 8], F32, isOutput=False)
    yd = nc.declare_dram_parameter("y", [shard, 8], BF16, isOutput=True)

    with TileContext(nc) as tc:
        with (
            tc.tile_pool(name="stream", bufs=2) as stream,
            tc.tile_pool(name="work", bufs=2) as work,
            tc.tile_pool(name="small", bufs=2) as small,
        ):
            for _ in range(reps):
                for t in range(ntiles):
                    _emit_tile(nc, (stream, work, small), xd, yd, t, tf)
    nc.finalize()
    return nc


_BUILD_CACHE = {}
_RUNNER_CACHE = {}


def _get_runner(shard, tf):
    key = (shard, tf)
    if key not in _RUNNER_CACHE:
        import jax
        import jax.numpy as jnp
        from jax.experimental.shard_map import shard_map
        from jax.sharding import Mesh, NamedSharding, PartitionSpec
        from concourse.bass2jax import (
            _bass_exec_p,
            install_neuronx_cc_hook,
            partition_id_tensor,
        )

        install_neuronx_cc_hook()
        if key not in _BUILD_CACHE:
            _BUILD_CACHE[key] = build_nc(shard, tf)
        nc = _BUILD_CACHE[key]

        partition_name = (
            nc.partition_id_tensor.name if nc.partition_id_tensor else None
        )
        in_names, out_names, out_avals = [], [], []
        for alloc in nc.m.functions[0].allocations:
            if not isinstance(alloc, mybir.MemoryLocationSet):
                continue
            name = alloc.memorylocations[0].name
            if alloc.kind == "ExternalInput":
                if name != partition_name:
                    in_names.append(name)
            elif alloc.kind == "ExternalOutput":
                out_names.append(name)
                out_avals.append(
                    jax.core.ShapedArray(
                        tuple(alloc.tensor_shape), mybir.dt.np(alloc.dtype)
                    )
                )
        n_params = len(in_names)
        all_in = list(in_names) + list(out_names)
        if partition_name is not None:
            all_in.append(partition_name)

        def _body(*args):
            operands = list(args)
            if partition_name is not None:
                operands.append(partition_id_tensor())
            outs = _bass_exec_p.bind(
                *operands,
                out_avals=tuple(out_avals),
                in_names=tuple(all_in),
                out_names=tuple(out_names),
                lowering_input_output_aliases=(),
                sim_require_finite=True,
                sim_require_nnan=True,
                nc=nc,
            )
            return tuple(outs)

        devices = jax.devices()[:N_CORES]
        mesh = Mesh(np.asarray(devices), ("core",))
        spec = PartitionSpec("core")
        sharding = NamedSharding(mesh, spec)
        in_specs = (spec,) * (n_params + len(out_names))
        out_specs = (spec,) * len(out_names)
        fn = jax.jit(
            shard_map(
                _body, mesh=mesh, in_specs=in_specs, out_specs=out_specs, check_rep=False
            ),
            donate_argnums=tuple(range(n_params, n_params + len(out_names))),
            keep_unused=True,
        )
        zero_fns = []
        for aval in out_avals:
            shape = (N_CORES * aval.shape[0],) + tuple(aval.shape[1:])
            zero_fns.append(
                jax.jit(
                    lambda shape=shape, dtype=aval.dtype: jnp.zeros(shape, dtype),
                    out_shardings=sharding,
                )
            )
        _RUNNER_CACHE[key] = (fn, zero_fns, sharding)
    return _RUNNER_CACHE[key]


def kernel(x: np.ndarray) -> np.ndarray:
    import jax

    x = np.ascontiguousarray(x, dtype=np.float32)
    n = x.shape[0]
    shard = n // N_CORES
    tf = TF
    while shard % (128 * tf) != 0:
        tf //= 2
    fn, zero_fns, sharding = _get_runner(shard, tf)
    xdev = jax.device_put(x, sharding)
    zeros = [zf() for zf in zero_fns]
    (ybf,) = fn(xdev, *zeros)
    return np.asarray(ybf).astype(np.float32)


# revision 12
# speedup vs baseline: 1.7393x; 1.7393x over previous
"""Nearest-E8-lattice quantizer (CachedE8Quantizer) as a Bass/Tile kernel on 8 trn2 cores.

Input x: [8388608, 8] fp32. Output: nearest point of E8 = D8 u (D8 + 1/2).

Sharding: data-parallel over the points dim, 1/8 per core (no comms).

Per-core pipeline, layout [128 partitions, TF points, 8 coords] per tile:
  r0  = round(x)            via (x + 1.5*2^23) - 1.5*2^23   (exact RNE)   [GP]
  r1h = round(x - 0.5)+0.5  via ((x-0.5)+C) - C + 0.5                     [GP]
  d_b = x - r_b   (exact, Sterbenz)                                       [DVE]
  q_b = d_b^2                                                             [ACT]
  segmented (per 8) reduces: s_b = sum r_b, S2_b = sum q_b, mq_b = max q_b [DVE]
  parity p_b of s_b; u_b = 1 - 2*sqrt(mq_b); D_b = S2_b + p_b*u_b;
  c = D0 <= D1; w_b = p_b * (c match); mgq_b = mq_b*w + (w-1)   (q or -1) [smalls]
  onehot = (q_b == mgq_b)  fp equality (argmax coord; rare sq-ties double-flip) [GP]
  flip_b = signbit(d_b) | onehot-bits   (+-1.0f at argmax, +-0.0 elsewhere) [DVE]
  y = (x - (c ? d0 : d1)) + flip0 + flip1                                  [DVE+GP]
"""

import numpy as np

from concourse import bacc
import concourse.mybir as mybir
from concourse.alu_op_type import AluOpType as op
from concourse.tile import TileContext

N_POINTS = 8388608
N_CORES = 8
SHARD = N_POINTS // N_CORES  # 1048576 points per core

MAGIC = 12582912.0  # 1.5 * 2**23: (x + MAGIC) - MAGIC == round-half-even(x)
F32 = mybir.dt.float32
U32 = mybir.dt.uint32
X = mybir.AxisListType.X

TF = 256


def _stt_u32(eng, out, in0, scalar_int, in1, op0, op1):
    """scalar_tensor_tensor with a uint32 immediate (bass default lowers ints
    as f32 immediates, which walrus rejects for bitvec ops)."""
    return eng.add_instruction(
        mybir.InstTensorScalarPtr(
            name=eng.bass.get_next_instruction_name(),
            is_scalar_tensor_tensor=True,
            op0=op0,
            op1=op1,
            ins=[
                eng.lower_ap(in0),
                mybir.ImmediateValue(dtype=U32, value=scalar_int),
                eng.lower_ap(in1),
            ],
            outs=[eng.lower_ap(out)],
        )
    )


ENGINES = {"round": "vector", "flsum": "vector", "delta": "vector", "round_act": False, "pe_y": True, "pe_d": False, "pe_qd": False}


def _emit_tile(nc, pools, xd, yd, t, tf):
    E = lambda k: getattr(nc, ENGINES[k])
    P = 128
    pts = P * tf
    FE = tf * 8
    stream, work, small = pools[:3]

    s = t * pts
    x_rows = xd[s : s + pts, :].rearrange("(p f) c -> p (f c)", p=P)
    y_rows = yd[s : s + pts, :].rearrange("(p f) c -> p (f c)", p=P)

    xt = stream.tile([P, FE], F32, tag="xt")
    nc.sync.dma_start(out=xt[:], in_=x_rows)

    # roundings on GPSIMD; t1 scratch shares the ohh slot
    rr = work.tile([P, 2 * FE], F32, tag="rr")
    t1 = work.tile([P, 2 * FE], F32, tag="ohh")
    r0, r1h = rr[:, :FE], rr[:, FE:]
    CP = mybir.ActivationFunctionType.Copy
    if ENGINES["round_act"]:
        nc.scalar.activation(r0, xt[:], CP, bias=MAGIC)
        nc.scalar.activation(r0, r0, CP, bias=-MAGIC)
        nc.scalar.activation(t1[:, :FE], xt[:], CP, bias=-0.5)
        nc.scalar.activation(t1[:, :FE], t1[:, :FE], CP, bias=MAGIC)
        nc.scalar.activation(r1h, t1[:, :FE], CP, bias=-MAGIC)
        nc.scalar.activation(r1h, r1h, CP, bias=0.5)
    else:
        E("round").tensor_scalar(r0, xt[:], MAGIC, MAGIC, op0=op.add, op1=op.subtract)
        E("round").tensor_scalar(t1[:, :FE], xt[:], 0.5, MAGIC, op0=op.subtract, op1=op.add)
        E("round").tensor_scalar(r1h, t1[:, :FE], MAGIC, 0.5, op0=op.subtract, op1=op.add)

    # deltas
    dd = work.tile([P, 2 * FE], F32, tag="dd")
    d0, d1 = dd[:, :FE], dd[:, FE:]
    if ENGINES["pe_d"]:
        psum_pool, ident, nident = pools[3]
        NCH = 512
        dp = psum_pool.tile([P, 2 * FE], F32, tag="dp")
        for c0 in range(0, FE, NCH):
            nc.tensor.matmul(dp[:, c0:c0+NCH], ident[:], xt[:, c0:c0+NCH], start=True, stop=False)
            nc.tensor.matmul(dp[:, c0:c0+NCH], nident[:], rr[:, c0:c0+NCH], start=False, stop=True)
            nc.tensor.matmul(dp[:, FE+c0:FE+c0+NCH], ident[:], xt[:, c0:c0+NCH], start=True, stop=False)
            nc.tensor.matmul(dp[:, FE+c0:FE+c0+NCH], nident[:], rr[:, FE+c0:FE+c0+NCH], start=False, stop=True)
        nc.scalar.copy(dd[:, :FE], dp[:, :FE])
        nc.scalar.copy(dd[:, FE:], dp[:, FE:])
    else:
        E("delta").tensor_tensor(d0, xt[:], r0, op.subtract)
        E("delta").tensor_tensor(d1, xt[:], r1h, op.subtract)
    dd_u = dd[:].bitcast(U32)

    # squares (ACT)
    qq = work.tile([P, 2 * FE], F32, tag="qq")
    nc.scalar.square(qq[:, :FE], d0)
    nc.scalar.square(qq[:, FE:], d1)
    qq3 = qq[:].rearrange("p (t c) -> p t c", c=8)

    # segmented reduces (DVE)
    rr3 = rr[:].rearrange("p (t c) -> p t c", c=8)
    TW = 2 * tf
    arena = small.tile([P, 8 * TW + tf], F32, tag="arena")
    savg = arena[:, 0 * TW : 1 * TW]
    qavg = arena[:, 1 * TW : 2 * TW]
    mq2 = arena[:, 2 * TW : 3 * TW]
    ps2 = arena[:, 3 * TW : 4 * TW]
    p2f = arena[:, 4 * TW : 5 * TW]
    u2 = arena[:, 5 * TW : 6 * TW]
    Dv2 = arena[:, 6 * TW : 7 * TW]  # also reused as vg scratch
    wf2 = arena[:, 7 * TW : 8 * TW]
    cf = arena[:, 8 * TW : 8 * TW + tf]
    nc.vector.tensor_reduce(savg, rr3, axis=X, op=op.add)
    if ENGINES["pe_qd"]:
        psum_pool2, ident2, nident2 = pools[3]
        NCH = 512
        qdp = psum_pool2.tile([P, FE], F32, tag="qdp")
        for c0 in range(0, FE, NCH):
            nc.tensor.matmul(qdp[:, c0:c0+NCH], ident2[:], qq[:, c0:c0+NCH], start=True, stop=False)
            nc.tensor.matmul(qdp[:, c0:c0+NCH], nident2[:], qq[:, FE+c0:FE+c0+NCH], start=False, stop=True)
        qdp3 = qdp[:].rearrange("p (t c) -> p t c", c=8)
        nc.vector.tensor_reduce(qavg[:, :tf], qdp3, axis=X, op=op.add)  # dS = S2_0 - S2_1
    else:
        nc.vector.tensor_reduce(qavg, qq3, axis=X, op=op.add)
    nc.vector.tensor_reduce(mq2, qq3, axis=X, op=op.max)

    # parity: ps = 2*round(s/2) - s in {-1,0,1}; p2f = ps^2 in {0,1}
    nc.vector.tensor_scalar(ps2, savg, 0.5, MAGIC, op0=op.mult, op1=op.add)
    nc.vector.tensor_scalar(ps2, ps2, MAGIC, None, op0=op.subtract)
    nc.vector.scalar_tensor_tensor(ps2, ps2, 2.0, savg, op0=op.mult, op1=op.subtract)
    nc.scalar.square(p2f, ps2)
    # u = 1 - 2*sqrt(mq)  (~1ulp sqrt; only perturbs borderline D compares)
    nc.scalar.sqrt(u2, mq2)
    nc.scalar.activation(
        u2, u2, mybir.ActivationFunctionType.Copy, bias=1.0, scale=-2.0
    )
    # D = S2 + p*u ; c = (D0 <= D1)
    nc.vector.tensor_tensor(Dv2, p2f, u2, op.mult)
    if ENGINES["pe_qd"]:
        nc.vector.tensor_tensor(Dv2[:, :tf], Dv2[:, :tf], qavg[:, :tf], op.add)
        nc.vector.tensor_tensor(cf, Dv2[:, :tf], Dv2[:, tf:], op.is_le)
    else:
        nc.vector.tensor_tensor(Dv2, Dv2, qavg, op.add)
        nc.vector.tensor_tensor(cf, Dv2[:, :tf], Dv2[:, tf:], op.is_le)
    # w0 = p0*c ; w1 = p1*(1-c); gated max-sq: mgq = mq*w + (w-1)
    nc.vector.tensor_tensor(wf2[:, :tf], p2f[:, :tf], cf, op.mult)
    nc.vector.scalar_tensor_tensor(
        wf2[:, tf:], cf, 1.0, p2f[:, tf:], op0=op.subtract, op1=op.mult
    )
    nc.vector.tensor_scalar(wf2[:, tf:], wf2[:, tf:], -1.0, None, op0=op.mult)
    nc.vector.tensor_tensor(Dv2, mq2, wf2, op.mult)
    nc.vector.tensor_scalar(wf2, wf2, 1.0, None, op0=op.subtract)
    nc.vector.tensor_tensor(mq2, Dv2, wf2, op.add)

    # onehot (GP, fp equality on squares); flip = sign(d) | onehot-bits (DVE)
    mgq_b = mq2.unsqueeze(2).broadcast_to([P, 2 * tf, 8])
    ohf = work.tile([P, 2 * FE], F32, tag="ohh")
    ohf3 = ohf[:].rearrange("p (t c) -> p t c", c=8)
    nc.vector.tensor_tensor(ohf3, qq3, mgq_b, op.is_equal)
    ohf_u = ohf[:].bitcast(U32)
    _stt_u32(nc.vector, ohf_u, dd_u[:], 0x80000000, ohf_u, op.bitwise_and, op.bitwise_or)
    fl = ohf[:]

    # d_sel = c ? d0 : d1 (ACT copy + DVE predicated); flsum on GP
    dsel = work.tile([P, 2 * FE], F32, tag="rr")
    nc.scalar.copy(dsel[:, :FE], d1)
    cI_b = cf.bitcast(U32).unsqueeze(2).broadcast_to([P, tf, 8])
    nc.vector.copy_predicated(
        dsel[:, :FE].rearrange("p (t c) -> p t c", c=8),
        cI_b,
        dd[:, :FE].rearrange("p (t c) -> p t c", c=8),
    )
    if ENGINES["pe_y"]:
        psum_pool, ident, nident = pools[3]
        yp = psum_pool.tile([P, FE], F32, tag="yp")
        NCH = 512
        for c0 in range(0, FE, NCH):
            sl = slice(c0, c0 + NCH)
            nc.tensor.matmul(yp[:, sl], ident[:], xt[:, sl], start=True, stop=False)
            nc.tensor.matmul(yp[:, sl], nident[:], dsel[:, c0:c0+NCH], start=False, stop=False)
            nc.tensor.matmul(yp[:, sl], ident[:], fl[:, c0:c0+NCH], start=False, stop=False)
            nc.tensor.matmul(yp[:, sl], ident[:], fl[:, FE+c0:FE+c0+NCH], start=False, stop=True)
        yt = stream.tile([P, FE], F32, tag="yt")
        nc.scalar.copy(yt[:], yp[:])
        nc.sync.dma_start(out=y_rows, in_=yt[:])
    else:
        E("flsum").tensor_tensor(dsel[:, FE:], fl[:, :FE], fl[:, FE:], op.add)
        yt = stream.tile([P, FE], F32, tag="yt")
        nc.vector.tensor_tensor(yt[:], xt[:], dsel[:, :FE], op.subtract)
        nc.vector.tensor_tensor(yt[:], yt[:], dsel[:, FE:], op.add)
        nc.sync.dma_start(out=y_rows, in_=yt[:])


def build_nc(shard=SHARD, tf=None, reps=1):
    if tf is None:
        tf = TF
    P = 128
    pts = P * tf
    assert shard % pts == 0
    ntiles = shard // pts

    nc = bacc.Bacc("TRN2", target_bir_lowering=False, debug=False, num_devices=N_CORES)
    xd = nc.declare_dram_parameter("x", [shard, 8], F32, isOutput=False)
    yd = nc.declare_dram_parameter("y", [shard, 8], F32, isOutput=True)

    from concourse.masks import make_identity
    with TileContext(nc) as tc:
        with (
            tc.tile_pool(name="stream", bufs=2) as stream,
            tc.tile_pool(name="work", bufs=2) as work,
            tc.tile_pool(name="small", bufs=2) as small,
            tc.tile_pool(name="const", bufs=1) as cpool,
            tc.tile_pool(name="psum", bufs=2, space="PSUM") as psum_pool,
        ):
            pe = None
            if ENGINES["pe_y"]:
                ident = cpool.tile([P, P], F32, tag="ident")
                nident = cpool.tile([P, P], F32, tag="nident")
                make_identity(nc, ident[:])
                nc.scalar.activation(
                    nident[:], ident[:], mybir.ActivationFunctionType.Copy, scale=-1.0
                )
                pe = (psum_pool, ident, nident)
            for _ in range(reps):
                for t in range(ntiles):
                    _emit_tile(nc, (stream, work, small, pe), xd, yd, t, tf)
    nc.finalize()
    return nc


_BUILD_CACHE = {}
_RUNNER_CACHE = {}


def _get_runner(shard, tf):
    key = (shard, tf)
    if key not in _RUNNER_CACHE:
        import jax
        import jax.numpy as jnp
        from jax.experimental.shard_map import shard_map
        from jax.sharding import Mesh, NamedSharding, PartitionSpec
        from concourse.bass2jax import (
            _bass_exec_p,
            install_neuronx_cc_hook,
            partition_id_tensor,
        )

        install_neuronx_cc_hook()
        if key not in _BUILD_CACHE:
            _BUILD_CACHE[key] = build_nc(shard, tf)
        nc = _BUILD_CACHE[key]

        partition_name = (
            nc.partition_id_tensor.name if nc.partition_id_tensor else None
        )
        in_names, out_names, out_avals = [], [], []
        for alloc in nc.m.functions[0].allocations:
            if not isinstance(alloc, mybir.MemoryLocationSet):
                continue
            name = alloc.memorylocations[0].name
            if alloc.kind == "ExternalInput":
                if name != partition_name:
                    in_names.append(name)
            elif alloc.kind == "ExternalOutput":
                out_names.append(name)
                out_avals.append(
                    jax.core.ShapedArray(
                        tuple(alloc.tensor_shape), mybir.dt.np(alloc.dtype)
                    )
                )
        n_params = len(in_names)
        all_in = list(in_names) + list(out_names)
        if partition_name is not None:
            all_in.append(partition_name)

        def _body(*args):
            operands = list(args)
            if partition_name is not None:
                operands.append(partition_id_tensor())
            outs = _bass_exec_p.bind(
                *operands,
                out_avals=tuple(out_avals),
                in_names=tuple(all_in),
                out_names=tuple(out_names),
                lowering_input_output_aliases=(),
                sim_require_finite=True,
                sim_require_nnan=True,
                nc=nc,
            )
            return tuple(outs)

        devices = jax.devices()[:N_CORES]
        mesh = Mesh(np.asarray(devices), ("core",))
        spec = PartitionSpec("core")
        sharding = NamedSharding(mesh, spec)
        in_specs = (spec,) * (n_params + len(out_names))
        out_specs = (spec,) * len(out_names)
        fn = jax.jit(
            shard_map(
                _body, mesh=mesh, in_specs=in_specs, out_specs=out_specs, check_rep=False
            ),
            donate_argnums=tuple(range(n_params, n_params + len(out_names))),
            keep_unused=True,
        )
        zero_fns = []
        for aval in out_avals:
            shape = (N_CORES * aval.shape[0],) + tuple(aval.shape[1:])
            zero_fns.append(
                jax.jit(
                    lambda shape=shape, dtype=aval.dtype: jnp.zeros(shape, dtype),
                    out_shardings=sharding,
                )
            )
        _RUNNER_CACHE[key] = (fn, zero_fns, sharding)
    return _RUNNER_CACHE[key]


def kernel(x: np.ndarray) -> np.ndarray:
    import jax

    x = np.ascontiguousarray(x, dtype=np.float32)
    n = x.shape[0]
    shard = n // N_CORES
    tf = TF
    while shard % (128 * tf) != 0:
        tf //= 2
    fn, zero_fns, sharding = _get_runner(shard, tf)
    xdev = jax.device_put(x, sharding)
    zeros = [zf() for zf in zero_fns]
    (yout,) = fn(xdev, *zeros)
    return np.asarray(yout)
